# revision 10
# baseline (speedup 1.0000x reference)
"""Trainium2 Bass kernel for MartingaleAwareAttention.

Math: the reference runs standard attention plus 20 permutation passes.
Full bidirectional attention with per-token projections is permutation-
equivariant, so each pass (permute -> attend -> unpermute) equals standard
attention exactly in real arithmetic. Hence

    out = (1-a)*std + a*var_w*perm_out = c * (attend(x) @ wo + bo),
    c = (1-a) + a*var_w,  a = clip(len_w * log(S)/S, 0.01, 1.0)

whenever perms are true permutations (checked at runtime; exact numpy
fallback otherwise).

Sharding: 8 cores = 4 batches x 2 head-halves (8 heads / 512 cols each).
Each core computes its half's attention and the partial @ wo[rows] product;
host sums the two halves, adds bo, scales by c.

Layout note: all per-core inputs are packed into 8 DRAM "chunk" tensors of
[128, 3072] so that each fp32r matmul reads both operands from a single
DMA'd tile -- the self-loading fp32r Matmult has a single sync-wait slot,
so any matmul needing two unobserved semaphore ticks fails walrus codegen.

chunk[dc] columns (f32r):
    0:512     xT rows dc*128..dc*128+127        (x[b].T chunk)
    512:1024  wq[:, half] rows dc*128..+128
    1024:1536 wk slice rows
    1536:2048 wv slice rows
    2048:3072 dc<4:  wo slice rows dc*128..+128  ([128, 1024])
              dc==4: cols 2048:2560 = bv broadcast [128, 512]
              dc==5: cols 2048:2560 = hmap pad    [128, 512]
              dc==6: 2048:2052 bqr, 2052:2056 bkr, 2056:2120 oneh
"""

import math

import numpy as np

B = 4
S = 512
DM = 1024
NHL = 8        # local heads per core
HD = 64
JL = NHL * HD  # 512 local head-dim columns
SCALE = HD ** -0.5
NCORES = 8
CW = 3072      # chunk width

_PROG = None


def _build_program(debug=False):
    import concourse.bacc as bacc
    import concourse.tile as tile
    from concourse import mybir

    f32 = mybir.dt.float32
    f32r = mybir.dt.float32r
    EXP = mybir.ActivationFunctionType.Exp
    COPY = mybir.ActivationFunctionType.Copy

    nc = bacc.Bacc()

    dbg = {}
    if debug:
        for nm, shp in [("dbgQT", [128, S]), ("dbgKT", [128, S]),
                        ("dbgV", [128, 8 * 65]), ("dbgPT", [128, S]),
                        ("dbgRD", [65, S]), ("dbgRSB", [64, S]),
                        ("dbgON", [128, S])]:
            dbg[nm] = nc.declare_dram_parameter(nm, shp, f32, isOutput=True)

    chunks = [
        nc.declare_dram_parameter(f"chunk{dc}", [128, CW], f32r, isOutput=False)
        for dc in range(8)
    ]
    out = nc.declare_dram_parameter("out", [S, DM], f32, isOutput=True)

    with tile.TileContext(nc) as tc:
        from contextlib import ExitStack

        with ExitStack() as ctx:
            wts = ctx.enter_context(tc.tile_pool(name="wts", bufs=1))
            ppt = ctx.enter_context(tc.tile_pool(name="ppt", bufs=8))

            # ---------------- input DMA: one per chunk ----------------
            ch = []
            for dc in range(8):
                t = wts.tile([128, CW], f32r, tag=f"ch{dc}", name=f"ch{dc}")
                nc.sync.dma_start(out=t, in_=chunks[dc][:, :])
                ch.append(t)

            def xT(dc):
                return ch[dc][:, 0:512]

            def wqc(dc):
                return ch[dc][:, 512:1024]

            def wkc(dc):
                return ch[dc][:, 1024:1536]

            def wvc(dc):
                return ch[dc][:, 1536:2048]

            def woc(jc):
                return ch[jc][:, 2048:3072]

            bvb = ch[4][:, 2048:2560].bitcast(f32)
            bqr = ch[6][:, 2048:2052].bitcast(f32)
            bkr = ch[6][:, 2052:2056].bitcast(f32)

            # ---------------- projections ----------------
            # Group A: V (out[s,dv]) and Q^T (out[j,s]); group B: K^T.
            QT = []
            KT = []
            V = []
            for st in range(4):
                t = wts.tile([128, 8 * 65], f32r, tag=f"V{st}", name=f"V{st}")
                nc.vector.memset(t.bitcast(f32), 1.0)
                V.append(t)
            ones1x64 = wts.tile([1, 64], f32r, tag="ones1x64", name="ones1x64")
            nc.vector.memset(ones1x64.bitcast(f32), 1.0)
            with tc.tile_pool(name="psP", bufs=1, space="PSUM") as psP:
                psv = []
                psq = []
                for st in range(4):
                    ps = psP.tile([128, JL], f32, tag="pa", bufs=8,
                                  name=f"ps_v{st}")
                    psv.append(ps)
                for jt in range(4):
                    ps = psP.tile([128, S], f32, tag="pa", bufs=8,
                                  name=f"ps_qt{jt}")
                    psq.append(ps)
                for dc in range(8):
                    for st in range(4):
                        nc.tensor.matmul(
                            psv[st],
                            xT(dc)[:, st * 128:(st + 1) * 128],
                            wvc(dc),
                            start=(dc == 0), stop=(dc == 7),
                        )
                    for jt in range(4):
                        nc.tensor.matmul(
                            psq[jt],
                            wqc(dc)[:, jt * 128:(jt + 1) * 128],
                            xT(dc),
                            start=(dc == 0), stop=(dc == 7),
                        )
                for st in range(4):
                    # V_aug[:, h*65:h*65+64] = psv + bv ; col 64 stays 1.0
                    nc.vector.tensor_add(
                        V[st].rearrange("p (h c) -> p h c", c=65)[:, :, 0:64],
                        psv[st].rearrange("p (h c) -> p h c", c=64),
                        bvb.rearrange("p (h c) -> p h c", c=64),
                    )
                for jt in range(4):
                    t = wts.tile([128, S], f32r, tag=f"QT{jt}", name=f"QT{jt}")
                    nc.vector.tensor_scalar_add(t, psq[jt], bqr[:, jt:jt + 1])
                    QT.append(t)
                # group B: K^T reuses freed slots
                psk = []
                for jt in range(4):
                    ps = psP.tile([128, S], f32, tag="pa", bufs=8,
                                  name=f"ps_kt{jt}")
                    psk.append(ps)
                for dc in range(8):
                    for jt in range(4):
                        nc.tensor.matmul(
                            psk[jt],
                            wkc(dc)[:, jt * 128:(jt + 1) * 128],
                            xT(dc),
                            start=(dc == 0), stop=(dc == 7),
                        )
                for jt in range(4):
                    t = wts.tile([128, S], f32r, tag=f"KT{jt}", name=f"KT{jt}")
                    nc.vector.tensor_scalar_add(t, psk[jt], bkr[:, jt:jt + 1])
                    KT.append(t)

            # ---------------- attention ----------------
            # Per head: scores^T -> exp -> O^T accumulation with V_aug
            # ([65, 512]: rows 0-63 = unnormalized O^T, row 64 = denom),
            # then per-head normalization via K=1 reciprocal broadcast.
            Onorm = []
            for pair in range(4):
                t = wts.tile([128, S], f32r, tag=f"On{pair}", name=f"On{pair}")
                Onorm.append(t)
            with tc.tile_pool(name="psA", bufs=1, space="PSUM") as psA:
                for pair in range(4):
                    for hh in range(2):
                        h = 2 * pair + hh
                        base = hh * 64
                        QTh = QT[pair][base:base + 64, :]
                        KTh = KT[pair][base:base + 64, :]
                        ps_o = psA.tile([65, S], f32, tag="o", bufs=2,
                                        name=f"ps_o{h}")
                        for kt in range(4):
                            ps_s = psA.tile([128, S], f32, tag="sc", bufs=2,
                                            name=f"ps_s{h}_{kt}")
                            nc.tensor.matmul(
                                ps_s,
                                KTh[:, kt * 128:(kt + 1) * 128],
                                QTh,
                                start=True, stop=True,
                            )
                            pt = ppt.tile([128, S], f32r, tag="pt", bufs=8,
                                          name=f"pt{h}_{kt}")
                            nc.scalar.activation(pt, ps_s, EXP, scale=SCALE)
                            if debug and h == 0 and kt == 0:
                                nc.sync.dma_start(out=dbg["dbgPT"][:, :],
                                                  in_=pt.bitcast(f32))
                            nc.tensor.matmul(
                                ps_o,
                                V[kt][:, h * 65:(h + 1) * 65],
                                pt,
                                start=(kt == 0), stop=(kt == 3),
                            )
                        # move O^T+denom to SBUF (base-0 op), shift the
                        # denom row to partition 0 via DMA, recip there
                        ou = wts.tile([65, S], f32, tag="ou", bufs=2,
                                      name=f"ou{h}")
                        nc.vector.tensor_copy(ou, ps_o)
                        dzero = wts.tile([1, S], f32, tag="dzero", bufs=2,
                                         name=f"dzero{h}")
                        nc.sync.dma_start(out=dzero, in_=ou[64:65, :])
                        rd = wts.tile([1, S], f32, tag="rd", bufs=2,
                                      name=f"rd{h}")
                        nc.vector.reciprocal_approx_fast(rd, dzero)
                        rdr = wts.tile([1, S], f32r, tag="rdr", bufs=2,
                                       name=f"rdr{h}")
                        nc.vector.tensor_copy(rdr, rd)
                        # broadcast recip row to 64 partitions via K=1 matmul
                        ps_r = psA.tile([64, S], f32, tag="r", bufs=2,
                                        name=f"ps_r{h}")
                        nc.tensor.matmul(
                            ps_r,
                            ones1x64,
                            rdr,
                            start=True, stop=True,
                        )
                        rsb = wts.tile([64, S], f32, tag="rsb", bufs=2,
                                       name=f"rsb{h}")
                        nc.vector.tensor_copy(rsb, ps_r)
                        if debug and h == 0:
                            nc.sync.dma_start(out=dbg["dbgRD"][0:1, :],
                                              in_=rd)
                            nc.sync.dma_start(out=dbg["dbgRSB"][:, :],
                                              in_=rsb)
                        if hh == 0:
                            nc.vector.tensor_mul(
                                Onorm[pair][0:64, :], rsb, ou[0:64, :])
                        else:
                            tmp = wts.tile([64, S], f32r, tag="tmpon", bufs=2,
                                           name=f"tmpon{h}")
                            nc.vector.tensor_mul(tmp, rsb, ou[0:64, :])
                            nc.sync.dma_start(
                                out=Onorm[pair][64:128, :], in_=tmp)

            if debug:
                nc.sync.dma_start(out=dbg["dbgQT"][:, :], in_=QT[0].bitcast(f32))
                nc.sync.dma_start(out=dbg["dbgKT"][:, :], in_=KT[0].bitcast(f32))
                nc.sync.dma_start(out=dbg["dbgV"][:, :], in_=V[0].bitcast(f32))
                nc.sync.dma_start(out=dbg["dbgON"][:, :],
                                  in_=Onorm[0].bitcast(f32))

            # ---------------- output projection ----------------
            with tc.tile_pool(name="psW", bufs=1, space="PSUM") as psW:
                for st in range(4):
                    osb = wts.tile([128, DM], f32, tag=f"osb{st}",
                                   name=f"osb{st}")
                    for mt in range(2):
                        ps_w = psW.tile([128, 512], f32, tag="wop", bufs=2,
                                        name=f"ps_w{st}_{mt}")
                        for jc in range(4):
                            nc.tensor.matmul(
                                ps_w,
                                Onorm[jc][:, st * 128:(st + 1) * 128],
                                woc(jc)[:, mt * 512:(mt + 1) * 512],
                                start=(jc == 0), stop=(jc == 3),
                            )
                        nc.scalar.activation(
                            osb[:, mt * 512:(mt + 1) * 512], ps_w, COPY)
                    nc.sync.dma_start(
                        out=out[st * 128:(st + 1) * 128, :], in_=osb)

    nc.compile()
    return nc


def _get_prog():
    global _PROG
    if _PROG is None:
        _PROG = _build_program()
    return _PROG


def _pack_chunks(xb, wq_s, wk_s, wv_s, wo_s, bq_s, bk_s, bv_s):
    """Build the 8 [128, CW] chunk arrays for one core."""
    xT = np.ascontiguousarray(xb.T)          # [1024, 512]
    chunks = []
    for dc in range(8):
        c = np.zeros((128, CW), np.float32)
        rs = slice(dc * 128, (dc + 1) * 128)
        c[:, 0:512] = xT[rs]
        c[:, 512:1024] = wq_s[rs]
        c[:, 1024:1536] = wk_s[rs]
        c[:, 1536:2048] = wv_s[rs]
        if dc < 4:
            c[:, 2048:3072] = wo_s[dc * 128:(dc + 1) * 128]
        elif dc == 4:
            c[:, 2048:2560] = bv_s[None, :]
        elif dc == 5:
            hm = np.zeros((NHL, 512), np.float32)
            for h in range(NHL):
                hm[h, h * HD:(h + 1) * HD] = 1.0
            c[0:NHL, 2048:2560] = hm
        elif dc == 6:
            c[:, 2048:2052] = bq_s.reshape(4, 128).T
            c[:, 2052:2056] = bk_s.reshape(4, 128).T
            oh = np.zeros((128, NHL * 8), np.float32)
            for h in range(NHL):
                oh[:, h * 8 + h] = 1.0
            c[:, 2056:2120] = oh
        chunks.append(c)
    return chunks


def _attend_np(x, wq, bq, wk, bk, wv, bv):
    Bn, Sn, D = x.shape
    H = D // HD

    def proj(w, b):
        return (x @ w + b).reshape(Bn, Sn, H, HD).transpose(0, 2, 1, 3)

    q, k, v = proj(wq, bq), proj(wk, bk), proj(wv, bv)
    s = np.einsum('bhqd,bhkd->bhqk', q, k) * (HD ** -0.5)
    s = s - s.max(axis=-1, keepdims=True)
    e = np.exp(s)
    attn = e / e.sum(axis=-1, keepdims=True)
    o = np.einsum('bhqk,bhkd->bhqd', attn, v)
    return o.transpose(0, 2, 1, 3).reshape(Bn, Sn, D)


def _numpy_fallback(x, wq, bq, wk, bk, wv, bv, wo, bo, var_w, len_w, perms):
    Sn = x.shape[1]
    standard = _attend_np(x, wq, bq, wk, bk, wv, bv) @ wo + bo
    acc = np.zeros_like(x)
    for p in perms:
        xp = np.take(x, p, axis=1)
        o = _attend_np(xp, wq, bq, wk, bk, wv, bv)
        inv = np.argsort(p)
        acc = acc + np.take(o, inv, axis=1)
    perm_out = (acc / perms.shape[0]) @ wo + bo
    adaptive = np.clip(len_w * (math.log(Sn) / Sn), 0.01, 1.0).astype(np.float32)
    return ((1.0 - adaptive) * standard + adaptive * var_w * perm_out).astype(
        np.float32)


def kernel(x, wq, bq, wk, bk, wv, bv, wo, bo, var_w, len_w, perms, **_kw):
    x = np.ascontiguousarray(np.asarray(x, dtype=np.float32))
    wq = np.asarray(wq, dtype=np.float32)
    bq = np.asarray(bq, dtype=np.float32)
    wk = np.asarray(wk, dtype=np.float32)
    bk = np.asarray(bk, dtype=np.float32)
    wv = np.asarray(wv, dtype=np.float32)
    bv = np.asarray(bv, dtype=np.float32)
    wo = np.asarray(wo, dtype=np.float32)
    bo = np.asarray(bo, dtype=np.float32)
    var_w = np.asarray(var_w, dtype=np.float32)
    len_w = np.asarray(len_w, dtype=np.float32)
    perms_np = np.asarray(perms)

    Sn = x.shape[1]
    idx = np.arange(Sn)
    if not all(np.array_equal(np.sort(p), idx) for p in perms_np):
        return _numpy_fallback(x, wq, bq, wk, bk, wv, bv, wo, bo,
                               var_w, len_w, perms_np)

    adaptive = np.clip(len_w * (math.log(Sn) / Sn), 0.01, 1.0).astype(np.float32)
    c = float(((1.0 - adaptive) + adaptive * var_w).reshape(-1)[0])

    in_maps = []
    for core in range(NCORES):
        b, g = divmod(core, 2)
        cs = slice(g * JL, (g + 1) * JL)
        chunks = _pack_chunks(x[b], wq[:, cs], wk[:, cs], wv[:, cs],
                              wo[cs, :], bq[cs], bk[cs], bv[cs])
        in_maps.append({f"chunk{dc}": chunks[dc] for dc in range(8)})

    from concourse.bass_utils import run_bass_kernel_spmd

    nc = _get_prog()
    res = run_bass_kernel_spmd(nc, in_maps, list(range(NCORES)))
    parts = [res.results[i]["out"] for i in range(NCORES)]

    outp = np.empty((B, Sn, DM), np.float32)
    for b in range(B):
        outp[b] = c * (parts[2 * b] + parts[2 * b + 1] + bo[None, :])
    return outp


# revision 12
# speedup vs baseline: 1.0736x; 1.0736x over previous
"""Trainium2 Bass kernel for MartingaleAwareAttention.

Math: the reference runs standard attention plus 20 permutation passes.
Full bidirectional attention with per-token projections is permutation-
equivariant, so each pass (permute -> attend -> unpermute) equals standard
attention exactly in real arithmetic. Hence

    out = (1-a)*std + a*var_w*perm_out = c * (attend(x) @ wo + bo),
    c = (1-a) + a*var_w,  a = clip(len_w * log(S)/S, 0.01, 1.0)

whenever perms are true permutations (checked at runtime; exact numpy
fallback otherwise).

Sharding: 8 cores = 4 batches x 2 head-halves (8 heads / 512 cols each).
Each core computes its half's attention and the partial @ wo[rows] product;
host sums the two halves, adds bo, scales by c.

Layout note: all per-core inputs are packed into 8 DRAM "chunk" tensors of
[128, 3072] so that each fp32r matmul reads both operands from a single
DMA'd tile -- the self-loading fp32r Matmult has a single sync-wait slot,
so any matmul needing two unobserved semaphore ticks fails walrus codegen.

chunk[dc] columns (f32r):
    0:512     xT rows dc*128..dc*128+127        (x[b].T chunk)
    512:1024  wq[:, half] rows dc*128..+128
    1024:1536 wk slice rows
    1536:2048 wv slice rows
    2048:3072 dc<4:  wo slice rows dc*128..+128  ([128, 1024])
              dc==4: cols 2048:2560 = bv broadcast [128, 512]
              dc==5: cols 2048:2560 = hmap pad    [128, 512]
              dc==6: 2048:2052 bqr, 2052:2056 bkr, 2056:2120 oneh
"""

import math

import numpy as np

B = 4
S = 512
DM = 1024
NHL = 8        # local heads per core
HD = 64
JL = NHL * HD  # 512 local head-dim columns
SCALE = HD ** -0.5
NCORES = 8
CW = 3072      # chunk width (dc 0-3, carries wo)
CW2 = 2560     # chunk width (dc 4-7)

_PROG = None


def _build_program(debug=False):
    import concourse.bacc as bacc
    import concourse.tile as tile
    from concourse import mybir

    f32 = mybir.dt.float32
    f32r = mybir.dt.float32r
    EXP = mybir.ActivationFunctionType.Exp
    COPY = mybir.ActivationFunctionType.Copy

    nc = bacc.Bacc()

    dbg = {}
    if debug:
        for nm, shp in [("dbgQT", [128, S]), ("dbgKT", [128, S]),
                        ("dbgV", [128, 8 * 65]), ("dbgPT", [128, S]),
                        ("dbgRD", [65, S]), ("dbgRSB", [64, S]),
                        ("dbgON", [128, S])]:
            dbg[nm] = nc.declare_dram_parameter(nm, shp, f32, isOutput=True)

    chunks = [
        nc.declare_dram_parameter(f"chunk{dc}", [128, CW if dc < 4 else CW2],
                                  f32r, isOutput=False)
        for dc in range(8)
    ]
    out = nc.declare_dram_parameter("out", [S, DM], f32, isOutput=True)

    with tile.TileContext(nc) as tc:
        from contextlib import ExitStack

        with ExitStack() as ctx:
            wts = ctx.enter_context(tc.tile_pool(name="wts", bufs=1))
            ppt = ctx.enter_context(tc.tile_pool(name="ppt", bufs=8))

            # ---------------- input DMA: one per chunk ----------------
            ch = []
            for dc in range(8):
                w = CW if dc < 4 else CW2
                t = wts.tile([128, w], f32r, tag=f"ch{dc}", name=f"ch{dc}")
                nc.sync.dma_start(out=t, in_=chunks[dc][:, :])
                ch.append(t)

            def xT(dc):
                return ch[dc][:, 0:512]

            def wqc(dc):
                return ch[dc][:, 512:1024]

            def wkc(dc):
                return ch[dc][:, 1024:1536]

            def wvc(dc):
                return ch[dc][:, 1536:2048]

            def woc(jc):
                return ch[jc][:, 2048:3072]

            bvb = ch[4][:, 2048:2560].bitcast(f32)
            bqr = ch[6][:, 2048:2052].bitcast(f32)
            bkr = ch[6][:, 2052:2056].bitcast(f32)

            # ---------------- projections ----------------
            # Group A: V (out[s,dv]) and Q^T (out[j,s]); group B: K^T.
            QT = []
            KT = []
            V = []
            for st in range(4):
                t = wts.tile([128, 8 * 65], f32r, tag=f"V{st}", name=f"V{st}")
                nc.vector.memset(t.bitcast(f32), 1.0)
                V.append(t)
            ones1x64 = wts.tile([1, 64], f32r, tag="ones1x64", name="ones1x64")
            nc.vector.memset(ones1x64.bitcast(f32), 1.0)
            with tc.tile_pool(name="psP", bufs=1, space="PSUM") as psP:
                psv = []
                psq = []
                for st in range(4):
                    ps = psP.tile([128, JL], f32, tag="pa", bufs=8,
                                  name=f"ps_v{st}")
                    psv.append(ps)
                for jt in range(4):
                    ps = psP.tile([128, S], f32, tag="pa", bufs=8,
                                  name=f"ps_qt{jt}")
                    psq.append(ps)
                for dc in range(8):
                    for st in range(4):
                        nc.tensor.matmul(
                            psv[st],
                            xT(dc)[:, st * 128:(st + 1) * 128],
                            wvc(dc),
                            start=(dc == 0), stop=(dc == 7),
                        )
                    for jt in range(4):
                        nc.tensor.matmul(
                            psq[jt],
                            wqc(dc)[:, jt * 128:(jt + 1) * 128],
                            xT(dc),
                            start=(dc == 0), stop=(dc == 7),
                        )
                for st in range(4):
                    # V_aug[:, h*65:h*65+64] = psv + bv ; col 64 stays 1.0
                    nc.vector.tensor_add(
                        V[st].rearrange("p (h c) -> p h c", c=65)[:, :, 0:64],
                        psv[st].rearrange("p (h c) -> p h c", c=64),
                        bvb.rearrange("p (h c) -> p h c", c=64),
                    )
                for jt in range(4):
                    t = wts.tile([128, S], f32r, tag=f"QT{jt}", name=f"QT{jt}")
                    nc.vector.tensor_scalar_add(t, psq[jt], bqr[:, jt:jt + 1])
                    QT.append(t)
                # group B: K^T reuses freed slots
                psk = []
                for jt in range(4):
                    ps = psP.tile([128, S], f32, tag="pa", bufs=8,
                                  name=f"ps_kt{jt}")
                    psk.append(ps)
                for dc in range(8):
                    for jt in range(4):
                        nc.tensor.matmul(
                            psk[jt],
                            wkc(dc)[:, jt * 128:(jt + 1) * 128],
                            xT(dc),
                            start=(dc == 0), stop=(dc == 7),
                        )
                for jt in range(4):
                    t = wts.tile([128, S], f32r, tag=f"KT{jt}", name=f"KT{jt}")
                    nc.vector.tensor_scalar_add(t, psk[jt], bkr[:, jt:jt + 1])
                    KT.append(t)

            # ---------------- attention ----------------
            # Pass 1 (per head): scores^T -> exp -> O^T accumulation with
            # V_aug ([65, 512]: rows 0-63 = unnorm O^T, row 64 = denom),
            # copy to SBUF, shift denom row to partition 0, reciprocal.
            # Pass 2 (per head): K=1 broadcast matmul + normalize multiply.
            # Splitting keeps the recip/DMA latency chain off the PE's
            # program order, so heads stream back-to-back on PE.
            Onorm = []
            for pair in range(4):
                t = wts.tile([128, S], f32r, tag=f"On{pair}", name=f"On{pair}")
                Onorm.append(t)
            ous = []
            rdrs = []
            with tc.tile_pool(name="psA", bufs=1, space="PSUM") as psA:
                for h in range(8):
                    pair, hh = divmod(h, 2)
                    base = hh * 64
                    QTh = QT[pair][base:base + 64, :]
                    KTh = KT[pair][base:base + 64, :]
                    ps_o = psA.tile([65, S], f32, tag="o", bufs=2,
                                    name=f"ps_o{h}")
                    for kt in range(4):
                        ps_s = psA.tile([128, S], f32, tag="sc", bufs=3,
                                        name=f"ps_s{h}_{kt}")
                        nc.tensor.matmul(
                            ps_s,
                            KTh[:, kt * 128:(kt + 1) * 128],
                            QTh,
                            start=True, stop=True,
                        )
                        pt = ppt.tile([128, S], f32r, tag="pt", bufs=8,
                                      name=f"pt{h}_{kt}")
                        nc.scalar.activation(pt, ps_s, EXP, scale=SCALE)
                        nc.tensor.matmul(
                            ps_o,
                            V[kt][:, h * 65:(h + 1) * 65],
                            pt,
                            start=(kt == 0), stop=(kt == 3),
                        )
                    ou = wts.tile([65, S], f32, tag=f"ou{h}", bufs=1,
                                  name=f"ou{h}")
                    nc.vector.tensor_copy(ou, ps_o)
                    ous.append(ou)
                    dzero = wts.tile([1, S], f32, tag="dzero", bufs=4,
                                     name=f"dzero{h}")
                    nc.sync.dma_start(out=dzero, in_=ou[64:65, :])
                    rd = wts.tile([1, S], f32, tag="rd", bufs=4,
                                  name=f"rd{h}")
                    nc.vector.reciprocal_approx_fast(rd, dzero)
                    rdr = wts.tile([1, S], f32r, tag=f"rdr{h}", bufs=1,
                                   name=f"rdr{h}")
                    nc.vector.tensor_copy(rdr, rd)
                    rdrs.append(rdr)
                    if debug and h == 0:
                        nc.sync.dma_start(out=dbg["dbgRD"][0:1, :], in_=rd)

                # pass 2: broadcast recip rows, normalize
                for h in range(8):
                    pair, hh = divmod(h, 2)
                    ps_r = psA.tile([64, S], f32, tag="r", bufs=2,
                                    name=f"ps_r{h}")
                    nc.tensor.matmul(
                        ps_r,
                        ones1x64,
                        rdrs[h],
                        start=True, stop=True,
                    )
                    rsb = wts.tile([64, S], f32, tag="rsb", bufs=2,
                                   name=f"rsb{h}")
                    nc.scalar.activation(rsb, ps_r, COPY)
                    if debug and h == 0:
                        nc.sync.dma_start(out=dbg["dbgRSB"][:, :], in_=rsb)
                    if hh == 0:
                        nc.vector.tensor_mul(
                            Onorm[pair][0:64, :], rsb, ous[h][0:64, :])
                    else:
                        tmp = wts.tile([64, S], f32r, tag="tmpon", bufs=2,
                                       name=f"tmpon{h}")
                        nc.vector.tensor_mul(tmp, rsb, ous[h][0:64, :])
                        nc.sync.dma_start(
                            out=Onorm[pair][64:128, :], in_=tmp)

            if debug:
                nc.sync.dma_start(out=dbg["dbgQT"][:, :], in_=QT[0].bitcast(f32))
                nc.sync.dma_start(out=dbg["dbgKT"][:, :], in_=KT[0].bitcast(f32))
                nc.sync.dma_start(out=dbg["dbgV"][:, :], in_=V[0].bitcast(f32))
                nc.sync.dma_start(out=dbg["dbgON"][:, :],
                                  in_=Onorm[0].bitcast(f32))

            # ---------------- output projection ----------------
            with tc.tile_pool(name="psW", bufs=1, space="PSUM") as psW:
                for st in range(4):
                    for mt in range(2):
                        ps_w = psW.tile([128, 512], f32, tag="wop", bufs=2,
                                        name=f"ps_w{st}_{mt}")
                        for jc in range(4):
                            nc.tensor.matmul(
                                ps_w,
                                Onorm[jc][:, st * 128:(st + 1) * 128],
                                woc(jc)[:, mt * 512:(mt + 1) * 512],
                                start=(jc == 0), stop=(jc == 3),
                            )
                        osb = wts.tile([128, 512], f32, tag="osb", bufs=3,
                                       name=f"osb{st}_{mt}")
                        nc.scalar.activation(osb, ps_w, COPY)
                        nc.sync.dma_start(
                            out=out[st * 128:(st + 1) * 128,
                                    mt * 512:(mt + 1) * 512],
                            in_=osb)

    nc.compile()
    return nc


def _get_prog():
    global _PROG
    if _PROG is None:
        _PROG = _build_program()
    return _PROG


def _pack_chunks(xb, wq_s, wk_s, wv_s, wo_s, bq_s, bk_s, bv_s):
    """Build the 8 [128, CW] chunk arrays for one core."""
    xT = np.ascontiguousarray(xb.T)          # [1024, 512]
    chunks = []
    for dc in range(8):
        c = np.zeros((128, CW if dc < 4 else CW2), np.float32)
        rs = slice(dc * 128, (dc + 1) * 128)
        c[:, 0:512] = xT[rs]
        c[:, 512:1024] = wq_s[rs]
        c[:, 1024:1536] = wk_s[rs]
        c[:, 1536:2048] = wv_s[rs]
        if dc < 4:
            c[:, 2048:3072] = wo_s[dc * 128:(dc + 1) * 128]
        elif dc == 4:
            c[:, 2048:2560] = bv_s[None, :]
        elif dc == 6:
            c[:, 2048:2052] = bq_s.reshape(4, 128).T
            c[:, 2052:2056] = bk_s.reshape(4, 128).T
        chunks.append(c)
    return chunks


def _attend_np(x, wq, bq, wk, bk, wv, bv):
    Bn, Sn, D = x.shape
    H = D // HD

    def proj(w, b):
        return (x @ w + b).reshape(Bn, Sn, H, HD).transpose(0, 2, 1, 3)

    q, k, v = proj(wq, bq), proj(wk, bk), proj(wv, bv)
    s = np.einsum('bhqd,bhkd->bhqk', q, k) * (HD ** -0.5)
    s = s - s.max(axis=-1, keepdims=True)
    e = np.exp(s)
    attn = e / e.sum(axis=-1, keepdims=True)
    o = np.einsum('bhqk,bhkd->bhqd', attn, v)
    return o.transpose(0, 2, 1, 3).reshape(Bn, Sn, D)


def _numpy_fallback(x, wq, bq, wk, bk, wv, bv, wo, bo, var_w, len_w, perms):
    Sn = x.shape[1]
    standard = _attend_np(x, wq, bq, wk, bk, wv, bv) @ wo + bo
    acc = np.zeros_like(x)
    for p in perms:
        xp = np.take(x, p, axis=1)
        o = _attend_np(xp, wq, bq, wk, bk, wv, bv)
        inv = np.argsort(p)
        acc = acc + np.take(o, inv, axis=1)
    perm_out = (acc / perms.shape[0]) @ wo + bo
    adaptive = np.clip(len_w * (math.log(Sn) / Sn), 0.01, 1.0).astype(np.float32)
    return ((1.0 - adaptive) * standard + adaptive * var_w * perm_out).astype(
        np.float32)


def kernel(x, wq, bq, wk, bk, wv, bv, wo, bo, var_w, len_w, perms, **_kw):
    x = np.ascontiguousarray(np.asarray(x, dtype=np.float32))
    wq = np.asarray(wq, dtype=np.float32)
    bq = np.asarray(bq, dtype=np.float32)
    wk = np.asarray(wk, dtype=np.float32)
    bk = np.asarray(bk, dtype=np.float32)
    wv = np.asarray(wv, dtype=np.float32)
    bv = np.asarray(bv, dtype=np.float32)
    wo = np.asarray(wo, dtype=np.float32)
    bo = np.asarray(bo, dtype=np.float32)
    var_w = np.asarray(var_w, dtype=np.float32)
    len_w = np.asarray(len_w, dtype=np.float32)
    perms_np = np.asarray(perms)

    Sn = x.shape[1]
    idx = np.arange(Sn)
    if not all(np.array_equal(np.sort(p), idx) for p in perms_np):
        return _numpy_fallback(x, wq, bq, wk, bk, wv, bv, wo, bo,
                               var_w, len_w, perms_np)

    adaptive = np.clip(len_w * (math.log(Sn) / Sn), 0.01, 1.0).astype(np.float32)
    c = float(((1.0 - adaptive) + adaptive * var_w).reshape(-1)[0])

    in_maps = []
    for core in range(NCORES):
        b, g = divmod(core, 2)
        cs = slice(g * JL, (g + 1) * JL)
        chunks = _pack_chunks(x[b], wq[:, cs], wk[:, cs], wv[:, cs],
                              wo[cs, :], bq[cs], bk[cs], bv[cs])
        in_maps.append({f"chunk{dc}": chunks[dc] for dc in range(8)})

    from concourse.bass_utils import run_bass_kernel_spmd

    nc = _get_prog()
    res = run_bass_kernel_spmd(nc, in_maps, list(range(NCORES)))
    parts = [res.results[i]["out"] for i in range(NCORES)]

    outp = np.empty((B, Sn, DM), np.float32)
    for b in range(B):
        outp[b] = c * (parts[2 * b] + parts[2 * b + 1] + bo[None, :])
    return outp


# revision 14
# speedup vs baseline: 1.3714x; 1.2773x over previous
"""Trainium2 Bass kernel for MartingaleAwareAttention.

Math: the reference runs standard attention plus 20 permutation passes.
Full bidirectional attention with per-token projections is permutation-
equivariant, so each pass (permute -> attend -> unpermute) equals standard
attention exactly in real arithmetic. Hence

    out = (1-a)*std + a*var_w*perm_out = c * (attend(x) @ wo + bo),
    c = (1-a) + a*var_w,  a = clip(len_w * log(S)/S, 0.01, 1.0)

whenever perms are true permutations (checked at runtime; exact numpy
fallback otherwise).

Sharding: 8 cores = 4 batches x 2 head-halves (8 heads / 512 cols each).
Each core computes its half's attention and the partial @ wo[rows] product;
host sums the two halves, adds bo, scales by c.

Device pipeline per core (all matmuls accumulate fp32 in PSUM):
  - inputs x^T | wq | wk | wv packed per 128-row d-chunk into one bf16
    "chunk" tensor each (section-ordered DMAs so Q/K projections start
    as soon as their sections land); wo kept separate in fp32r.
  - Q^T/K^T = wq/wk-chunk^T @ x^T-chunk  (bf16, accumulated over 8 chunks)
  - V_aug[s, h*65+c] = V with a ones column appended per head
  - per head: S^T = K^T_h-slice^T @ Q^T_h -> exp (ACT, scale=1/8) ->
    O^T accum = V_aug_h^T @ P^T  giving [65, 512]: rows 0-63 = unnorm
    O^T, row 64 = softmax denominator
  - denom row -> partition 0 via SBUF-SBUF DMA, reciprocal_approx_fast,
    K=1 ones-matmul broadcast to 64 partitions, multiply (all fp32/f32r)
  - partial = Onorm^T-chunks @ wo-rows (f32r), DMA out
"""

import math

import numpy as np

B = 4
S = 512
DM = 1024
NHL = 8        # local heads per core
HD = 64
JL = NHL * HD  # 512 local head-dim columns
SCALE = HD ** -0.5
NCORES = 8
CW = 2560      # bf16 chunk width: xT|wq|wk|wv (2048) + extras

_PROG = None


def _build_program():
    import concourse.bacc as bacc
    import concourse.tile as tile
    from concourse import mybir

    f32 = mybir.dt.float32
    f32r = mybir.dt.float32r
    bf16 = mybir.dt.bfloat16
    EXP = mybir.ActivationFunctionType.Exp
    COPY = mybir.ActivationFunctionType.Copy

    nc = bacc.Bacc()

    chunks = [
        nc.declare_dram_parameter(f"chunk{dc}", [128, CW], bf16, isOutput=False)
        for dc in range(8)
    ]
    wop = [
        nc.declare_dram_parameter(f"wo{jc}", [128, DM], f32r, isOutput=False)
        for jc in range(4)
    ]
    out = nc.declare_dram_parameter("out", [S, DM], f32, isOutput=True)

    with tile.TileContext(nc) as tc:
        from contextlib import ExitStack

        with ExitStack() as ctx:
            wts = ctx.enter_context(tc.tile_pool(name="wts", bufs=1))
            ppt = ctx.enter_context(tc.tile_pool(name="ppt", bufs=8))

            # ------------- input DMA, section-ordered -------------
            ch = []
            for dc in range(8):
                t = wts.tile([128, CW], bf16, tag=f"ch{dc}", name=f"ch{dc}")
                ch.append(t)
            # xT + wq sections first (Q projection), then wk, then wv+extras
            for dc in range(8):
                nc.sync.dma_start(out=ch[dc][:, 0:1024],
                                  in_=chunks[dc][:, 0:1024])
            for dc in range(8):
                nc.sync.dma_start(out=ch[dc][:, 1024:1536],
                                  in_=chunks[dc][:, 1024:1536])
            for dc in range(8):
                nc.sync.dma_start(out=ch[dc][:, 1536:CW],
                                  in_=chunks[dc][:, 1536:CW])
            wo_sb = []
            for jc in range(4):
                t = wts.tile([128, DM], f32r, tag=f"wo{jc}", name=f"wosb{jc}")
                nc.sync.dma_start(out=t, in_=wop[jc][:, :])
                wo_sb.append(t)

            def xT(dc):
                return ch[dc][:, 0:512]

            def wqc(dc):
                return ch[dc][:, 512:1024]

            def wkc(dc):
                return ch[dc][:, 1024:1536]

            def wvc(dc):
                return ch[dc][:, 1536:2048]

            bvb = ch[4][:, 2048:2560]
            # tensor_scalar needs an fp32 scalar operand; upcast the tiny
            # bias columns once
            bqr = wts.tile([128, 4], f32, tag="bqr", name="bqr")
            nc.vector.tensor_copy(bqr, ch[6][:, 2048:2052])
            bkr = wts.tile([128, 4], f32, tag="bkr", name="bkr")
            nc.vector.tensor_copy(bkr, ch[6][:, 2052:2056])

            # ------------- projections: QT+KT first, V second -------------
            QT = []
            KT = []
            V = []
            for st in range(4):
                t = wts.tile([128, 8 * 65], bf16, tag=f"V{st}", name=f"V{st}")
                nc.vector.memset(t, 1.0)
                V.append(t)
            ones1x64 = wts.tile([1, 64], f32r, tag="ones1x64", name="ones1x64")
            nc.vector.memset(ones1x64.bitcast(f32), 1.0)
            with tc.tile_pool(name="psP", bufs=1, space="PSUM") as psP:
                psq = []
                psk = []
                for jt in range(4):
                    psq.append(psP.tile([128, S], f32, tag="pa", bufs=8,
                                        name=f"ps_qt{jt}"))
                for jt in range(4):
                    psk.append(psP.tile([128, S], f32, tag="pa", bufs=8,
                                        name=f"ps_kt{jt}"))
                for dc in range(8):
                    for jt in range(4):
                        nc.tensor.matmul(
                            psq[jt],
                            wqc(dc)[:, jt * 128:(jt + 1) * 128],
                            xT(dc),
                            start=(dc == 0), stop=(dc == 7),
                        )
                    for jt in range(4):
                        nc.tensor.matmul(
                            psk[jt],
                            wkc(dc)[:, jt * 128:(jt + 1) * 128],
                            xT(dc),
                            start=(dc == 0), stop=(dc == 7),
                        )
                for jt in range(4):
                    t = wts.tile([128, S], bf16, tag=f"QT{jt}", name=f"QT{jt}")
                    nc.vector.tensor_scalar_add(t, psq[jt], bqr[:, jt:jt + 1])
                    QT.append(t)
                for jt in range(4):
                    t = wts.tile([128, S], bf16, tag=f"KT{jt}", name=f"KT{jt}")
                    nc.vector.tensor_scalar_add(t, psk[jt], bkr[:, jt:jt + 1])
                    KT.append(t)
                # V group reuses freed slots
                psv = []
                for st in range(4):
                    psv.append(psP.tile([128, JL], f32, tag="pa", bufs=8,
                                        name=f"ps_v{st}"))
                for dc in range(8):
                    for st in range(4):
                        nc.tensor.matmul(
                            psv[st],
                            xT(dc)[:, st * 128:(st + 1) * 128],
                            wvc(dc),
                            start=(dc == 0), stop=(dc == 7),
                        )
                for st in range(4):
                    nc.vector.tensor_add(
                        V[st].rearrange("p (h c) -> p h c", c=65)[:, :, 0:64],
                        psv[st].rearrange("p (h c) -> p h c", c=64),
                        bvb.rearrange("p (h c) -> p h c", c=64),
                    )

            # ------------- attention -------------
            # Pass 1 streams both heads of each pair on PE; the recip/DMA
            # latency chain runs on DVE/DMA off the PE program order.
            Onorm = []
            for pair in range(4):
                t = wts.tile([128, S], f32r, tag=f"On{pair}", name=f"On{pair}")
                Onorm.append(t)
            ous = []
            rdrs = []
            with tc.tile_pool(name="psA", bufs=1, space="PSUM") as psA:
                for pair in range(4):
                    po = []
                    for hh in range(2):
                        po.append(psA.tile([65, S], f32, tag="o", bufs=2,
                                           name=f"ps_o{2 * pair + hh}"))
                    for kt in range(4):
                        pts = []
                        for hh in range(2):
                            h = 2 * pair + hh
                            base = hh * 64
                            ps_s = psA.tile([128, S], f32, tag="sc", bufs=4,
                                            name=f"ps_s{h}_{kt}")
                            nc.tensor.matmul(
                                ps_s,
                                KT[pair][base:base + 64,
                                         kt * 128:(kt + 1) * 128],
                                QT[pair][base:base + 64, :],
                                start=True, stop=True,
                            )
                            pt = ppt.tile([128, S], bf16, tag="pt", bufs=8,
                                          name=f"pt{h}_{kt}")
                            nc.scalar.activation(pt, ps_s, EXP, scale=SCALE)
                            pts.append(pt)
                        for hh in range(2):
                            h = 2 * pair + hh
                            nc.tensor.matmul(
                                po[hh],
                                V[kt][:, h * 65:(h + 1) * 65],
                                pts[hh],
                                start=(kt == 0), stop=(kt == 3),
                            )
                    for hh in range(2):
                        h = 2 * pair + hh
                        ou = wts.tile([65, S], f32, tag=f"ou{h}", bufs=1,
                                      name=f"ou{h}")
                        nc.vector.tensor_copy(ou, po[hh])
                        ous.append(ou)
                        dzero = wts.tile([1, S], f32, tag="dzero", bufs=4,
                                         name=f"dzero{h}")
                        nc.sync.dma_start(out=dzero, in_=ou[64:65, :])
                        rd = wts.tile([1, S], f32, tag="rd", bufs=4,
                                      name=f"rd{h}")
                        nc.vector.reciprocal_approx_fast(rd, dzero)
                        rdr = wts.tile([1, S], f32r, tag=f"rdr{h}", bufs=1,
                                       name=f"rdr{h}")
                        nc.vector.tensor_copy(rdr, rd)
                        rdrs.append(rdr)

                # pass 2: broadcast recip rows, normalize
                for h in range(8):
                    pair, hh = divmod(h, 2)
                    ps_r = psA.tile([64, S], f32, tag="r", bufs=2,
                                    name=f"ps_r{h}")
                    nc.tensor.matmul(
                        ps_r,
                        ones1x64,
                        rdrs[h],
                        start=True, stop=True,
                    )
                    rsb = wts.tile([64, S], f32, tag="rsb", bufs=2,
                                   name=f"rsb{h}")
                    nc.scalar.activation(rsb, ps_r, COPY)
                    if hh == 0:
                        nc.vector.tensor_mul(
                            Onorm[pair][0:64, :], rsb, ous[h][0:64, :])
                    else:
                        tmp = wts.tile([64, S], f32r, tag="tmpon", bufs=2,
                                       name=f"tmpon{h}")
                        nc.vector.tensor_mul(tmp, rsb, ous[h][0:64, :])
                        nc.sync.dma_start(
                            out=Onorm[pair][64:128, :], in_=tmp)

            # ------------- output projection -------------
            with tc.tile_pool(name="psW", bufs=1, space="PSUM") as psW:
                for st in range(4):
                    for mt in range(2):
                        ps_w = psW.tile([128, 512], f32, tag="wop", bufs=2,
                                        name=f"ps_w{st}_{mt}")
                        for jc in range(4):
                            nc.tensor.matmul(
                                ps_w,
                                Onorm[jc][:, st * 128:(st + 1) * 128],
                                wo_sb[jc][:, mt * 512:(mt + 1) * 512],
                                start=(jc == 0), stop=(jc == 3),
                            )
                        osb = wts.tile([128, 512], f32, tag="osb", bufs=3,
                                       name=f"osb{st}_{mt}")
                        nc.scalar.activation(osb, ps_w, COPY)
                        nc.sync.dma_start(
                            out=out[st * 128:(st + 1) * 128,
                                    mt * 512:(mt + 1) * 512],
                            in_=osb)

    nc.compile()
    return nc


def _get_prog():
    global _PROG
    if _PROG is None:
        _PROG = _build_program()
    return _PROG


def _pack_chunks(xb, wq_s, wk_s, wv_s, bq_s, bk_s, bv_s, bf16):
    """Build the 8 [128, CW] bf16 chunk arrays for one core."""
    xT = np.ascontiguousarray(xb.T)          # [1024, 512]
    chunks = []
    for dc in range(8):
        c = np.zeros((128, CW), np.float32)
        rs = slice(dc * 128, (dc + 1) * 128)
        c[:, 0:512] = xT[rs]
        c[:, 512:1024] = wq_s[rs]
        c[:, 1024:1536] = wk_s[rs]
        c[:, 1536:2048] = wv_s[rs]
        if dc == 4:
            c[:, 2048:2560] = bv_s[None, :]
        elif dc == 6:
            c[:, 2048:2052] = bq_s.reshape(4, 128).T
            c[:, 2052:2056] = bk_s.reshape(4, 128).T
        chunks.append(c.astype(bf16))
    return chunks


def _attend_np(x, wq, bq, wk, bk, wv, bv):
    Bn, Sn, D = x.shape
    H = D // HD

    def proj(w, b):
        return (x @ w + b).reshape(Bn, Sn, H, HD).transpose(0, 2, 1, 3)

    q, k, v = proj(wq, bq), proj(wk, bk), proj(wv, bv)
    s = np.einsum('bhqd,bhkd->bhqk', q, k) * (HD ** -0.5)
    s = s - s.max(axis=-1, keepdims=True)
    e = np.exp(s)
    attn = e / e.sum(axis=-1, keepdims=True)
    o = np.einsum('bhqk,bhkd->bhqd', attn, v)
    return o.transpose(0, 2, 1, 3).reshape(Bn, Sn, D)


def _numpy_fallback(x, wq, bq, wk, bk, wv, bv, wo, bo, var_w, len_w, perms):
    Sn = x.shape[1]
    standard = _attend_np(x, wq, bq, wk, bk, wv, bv) @ wo + bo
    acc = np.zeros_like(x)
    for p in perms:
        xp = np.take(x, p, axis=1)
        o = _attend_np(xp, wq, bq, wk, bk, wv, bv)
        inv = np.argsort(p)
        acc = acc + np.take(o, inv, axis=1)
    perm_out = (acc / perms.shape[0]) @ wo + bo
    adaptive = np.clip(len_w * (math.log(Sn) / Sn), 0.01, 1.0).astype(np.float32)
    return ((1.0 - adaptive) * standard + adaptive * var_w * perm_out).astype(
        np.float32)


def kernel(x, wq, bq, wk, bk, wv, bv, wo, bo, var_w, len_w, perms, **_kw):
    x = np.ascontiguousarray(np.asarray(x, dtype=np.float32))
    wq = np.asarray(wq, dtype=np.float32)
    bq = np.asarray(bq, dtype=np.float32)
    wk = np.asarray(wk, dtype=np.float32)
    bk = np.asarray(bk, dtype=np.float32)
    wv = np.asarray(wv, dtype=np.float32)
    bv = np.asarray(bv, dtype=np.float32)
    wo = np.asarray(wo, dtype=np.float32)
    bo = np.asarray(bo, dtype=np.float32)
    var_w = np.asarray(var_w, dtype=np.float32)
    len_w = np.asarray(len_w, dtype=np.float32)
    perms_np = np.asarray(perms)

    Sn = x.shape[1]
    idx = np.arange(Sn)
    if not all(np.array_equal(np.sort(p), idx) for p in perms_np):
        return _numpy_fallback(x, wq, bq, wk, bk, wv, bv, wo, bo,
                               var_w, len_w, perms_np)

    import ml_dtypes
    bf16 = ml_dtypes.bfloat16

    adaptive = np.clip(len_w * (math.log(Sn) / Sn), 0.01, 1.0).astype(np.float32)
    c = float(((1.0 - adaptive) + adaptive * var_w).reshape(-1)[0])

    in_maps = []
    for core in range(NCORES):
        b, g = divmod(core, 2)
        cs = slice(g * JL, (g + 1) * JL)
        chunks = _pack_chunks(x[b], wq[:, cs], wk[:, cs], wv[:, cs],
                              bq[cs], bk[cs], bv[cs], bf16)
        m = {f"chunk{dc}": chunks[dc] for dc in range(8)}
        wo_s = np.ascontiguousarray(wo[cs, :])
        for jc in range(4):
            m[f"wo{jc}"] = np.ascontiguousarray(
                wo_s[jc * 128:(jc + 1) * 128, :])
        in_maps.append(m)

    from concourse.bass_utils import run_bass_kernel_spmd

    nc = _get_prog()
    res = run_bass_kernel_spmd(nc, in_maps, list(range(NCORES)))
    parts = [res.results[i]["out"] for i in range(NCORES)]

    outp = np.empty((B, Sn, DM), np.float32)
    for b in range(B):
        outp[b] = c * (parts[2 * b] + parts[2 * b + 1] + bo[None, :])
    return outp
